# revision 47
# baseline (speedup 1.0000x reference)
"""Multi-head attention (B=16, N=1024, E=768, H=12) on 8 TRN2 NeuronCores.

Data parallel over batch (2 per core, no collectives). Per-core fused kernel
with a "left-shifted" schedule (~349us vs the 375us phase-serial baseline):
  - The PE is the global bottleneck (~306us busy incl overheads; the Act
    engine's softmax Exp stream is ~225us). The baseline ran QKV then
    attention mostly serially (first exp at ~90us); here the exp stream
    starts at ~25us: the prefix emits only K0/Q0 window-0 psum groups, and
    ALL other projection work (46 QK groups, V chunks, batch-0 out-proj)
    becomes deadline-scheduled filler popped into the attention loop's tk
    slots, keeping the PE dense while Act's ~40us of slack absorbs the
    resulting exp-stream jitter.
  - Keeping the PE *dense* matters beyond overlap: gaps early in the run
    leave the PE at a reduced clock (512-col matmul 454ns vs 379ns);
    chunky fillers + warmup matmuls hold the full clock.
  - Subgroup loop is batch-major so batch-0's out-projection overlaps
    batch-1's attention. The last two subgroups run their own attn@V
    inline (lag 2 behind the exp stream), which pulls the final drains a
    subgroup earlier and leaves only ~12us of out-proj + DMA tail.
  - DMA rings are ~100GB/s each and issue ~600ns apiece, serialized per
    engine: prefix-critical loads (x batch-0 window-0 halves, K0/Q0
    strips) lead the sync/scalar rings; late-needed loads trail. Issuing
    dma_start from inside the compute loop RACES (nondeterministic
    corruption) - all DMAs are issued up front, ordered per ring.
Math/layouts per head match the baseline bit-for-bit (energy pairs as
concurrent row-tiled matmuls, exp with scale=1/8 and no max-subtraction,
[V|1] lhsT for fused denominators, shifted-duplicate DOUBLE layout for the
scrambled (H,N,D)->(N,E) reshape feeding clean K=128 out-proj matmuls).
"""

import contextlib

import numpy as np

import concourse.bass as bass
import concourse.tile as tile
import concourse.mybir as mybir
from concourse import bacc
from concourse import bass_utils

B, N, E, H = 16, 1024, 768, 12
D = E // H          # 64
N_CORES = 8
BPC = B // N_CORES  # 2
T = BPC * N         # 2048
F3 = 3 * E
SCALE = 1.0 / float(np.sqrt(np.float32(D)))

FP32 = mybir.dt.float32
BF16 = mybir.dt.bfloat16
AF = mybir.ActivationFunctionType
OP = mybir.AluOpType


def _emit(tc, x_ap, wqkv_ap, bqkv_ap, wout_ap, bout_ap, out_ap):
    nc = tc.nc
    EC = E // 128      # 6
    FC = 2 * E // 128  # 12
    HM = H * N         # 12288

    stack = contextlib.ExitStack()
    with stack:
        const_pool = stack.enter_context(tc.tile_pool(name="const", bufs=1))
        w_pool = stack.enter_context(tc.tile_pool(name="w", bufs=1))
        qkt_pool = stack.enter_context(tc.tile_pool(name="qkt", bufs=1))
        vo_pool = stack.enter_context(tc.tile_pool(name="vo", bufs=1))
        dbl_pool = stack.enter_context(tc.tile_pool(name="dbl", bufs=1))
        xt_pool = stack.enter_context(tc.tile_pool(name="xt", bufs=1))
        warm_pool = stack.enter_context(tc.tile_pool(name="warm", bufs=1))

        pse = stack.enter_context(
            tc.tile_pool(name="pse", bufs=2, space="PSUM"))   # (128,1024) = 2 banks
        pso = stack.enter_context(
            tc.tile_pool(name="pso", bufs=4, space="PSUM"))   # (65,512) = 1 bank

        # ---- PE warmup during DMA wait (p-state ramp ~3us). Emitted first
        # so the memset is the DVE queue's first op and the matmuls start
        # immediately (they have no DMA deps).
        warm = warm_pool.tile([128, 512], BF16, tag="warm")
        nc.vector.memset(warm[:, :], 0.0)
        for _ in range(8):
            wps = pse.tile([128, 1024], FP32, tag="pse")
            nc.tensor.matmul(wps[:, 0:512], warm[:, 0:128], warm[:, :],
                             start=True, stop=True)

        # ---- weight / x tiles -------------------------------------------
        wsb = [w_pool.tile([128, F3], BF16, tag=f"wsb{ec}", name=f"wsb{ec}")
               for ec in range(EC)]
        wosb = [w_pool.tile([128, E], BF16, tag=f"wosb{ec}", name=f"wosb{ec}")
                for ec in range(EC)]
        xt = [xt_pool.tile([128, T], BF16, tag=f"xt{ec}", name=f"xt{ec}")
              for ec in range(EC)]

        # Critical-path DMAs: the prefix only needs x window 0 (cols 0:512)
        # plus the K0/Q0 strips, so x batch 0 is loaded in 512-col halves.
        # scalar ring: K0/K1 strips, then Q0/Q1 strips.
        for ec in range(EC):
            nc.scalar.dma_start(wsb[ec][:, 768:1024],
                                wqkv_ap[ec * 128:(ec + 1) * 128, 768:1024])
        for ec in range(EC):
            nc.scalar.dma_start(wsb[ec][:, 0:256],
                                wqkv_ap[ec * 128:(ec + 1) * 128, 0:256])
        # sync ring: x batch-0 halves (window 0 first — gates the prefix).
        for ec in range(EC):
            nc.sync.dma_start(xt[ec][:, 0:512], x_ap[ec * 128:(ec + 1) * 128, 0:512])
        for ec in range(EC):
            nc.sync.dma_start(xt[ec][:, 512:N],
                              x_ap[ec * 128:(ec + 1) * 128, 512:N])
        # gpsimd: bias constants.
        bq = const_pool.tile([128, FC], FP32, tag="bq")
        nc.gpsimd.dma_start(bq[:, :], bqkv_ap.rearrange("(c p) -> p c", p=128)[:, 0:FC])
        bv_row = const_pool.tile([1, E], FP32, tag="brow", name="bv_row")
        nc.gpsimd.dma_start(bv_row[:, :], bqkv_ap[2 * E:3 * E].unsqueeze(0))
        bv = const_pool.tile([128, E], FP32, tag="bv")
        nc.gpsimd.partition_broadcast(bv[:, :], bv_row[:, :], channels=128)
        bo_row = const_pool.tile([1, E], FP32, tag="brow", name="bo_row")
        nc.gpsimd.dma_start(bo_row[:, :], bout_ap.unsqueeze(0))
        bo = const_pool.tile([128, E], FP32, tag="bo")
        nc.gpsimd.partition_broadcast(bo[:, :], bo_row[:, :], channels=128)
        # sync, in transfer-priority order (per-queue transfers run in issue
        # order, so late-needed loads go last and don't contend for HBM
        # early): V weights (due ~sub1), Q2-5/K2-5 strips (due sub 4+),
        # x batch 1 (due ~sub 8), out-proj weights (due ~sub 13).
        for ec in range(EC):
            nc.sync.dma_start(wsb[ec][:, 1536:2304],
                              wqkv_ap[ec * 128:(ec + 1) * 128, 1536:2304])
        for ec in range(EC):
            nc.sync.dma_start(wsb[ec][:, 256:768],
                              wqkv_ap[ec * 128:(ec + 1) * 128, 256:768])
            nc.sync.dma_start(wsb[ec][:, 1024:1536],
                              wqkv_ap[ec * 128:(ec + 1) * 128, 1024:1536])
        for ec in range(EC):
            nc.sync.dma_start(xt[ec][:, N:T], x_ap[ec * 128:(ec + 1) * 128, N:T])
        for ec in range(EC):
            nc.sync.dma_start(wosb[ec][:, :],
                              wout_ap[ec * 128:(ec + 1) * 128, :])

        # ---- Q/K production ---------------------------------------------
        qtiles, ktiles = {}, {}

        def emit_qk_psum(fci, kind, b, tql):
            """One psum group: Q or K chunk fci, batch b, 512-token window."""
            tiles = qtiles if kind == 0 else ktiles
            key = (b, fci)
            if key not in tiles:
                tiles[key] = qkt_pool.tile(
                    [128, N], BF16, tag=f"{'qk'[kind]}{fci % 3}",
                    name=f"{'qk'[kind]}t{b}_{fci}")
            fc = fci + 6 * kind
            tch = 2 * b + tql
            ps = pse.tile([128, 1024], FP32, tag="pse")
            for ec in range(EC):
                nc.tensor.matmul(
                    ps[:, 0:512],
                    wsb[ec][:, fc * 128:(fc + 1) * 128],
                    xt[ec][:, tch * 512:(tch + 1) * 512],
                    start=(ec == 0), stop=(ec == EC - 1))
            nc.vector.tensor_scalar_add(
                tiles[key][:, tql * 512:(tql + 1) * 512], ps[:, 0:512],
                bq[:, fc:fc + 1])

        # ---- V chunks (tok-major, ones col per head) --------------------
        vo = [vo_pool.tile([128, H * (D + 1)], BF16, tag=f"vo{i}",
                           name=f"vo{i}") for i in range(T // 128)]

        def emit_v_a(tc16):
            """First 8 heads' V for one 128-token chunk (+ ones cols)."""
            ps = pse.tile([128, 1024], FP32, tag="pse")
            for ec in range(EC):
                nc.tensor.matmul(
                    ps[:, 0:512],
                    xt[ec][:, tc16 * 128:(tc16 + 1) * 128],
                    wsb[ec][:, 2 * E:2 * E + 512],
                    start=(ec == 0), stop=(ec == EC - 1))
            nc.vector.memset(vo[tc16][:, D::(D + 1)], 1.0)
            vo3a = vo[tc16][:, 0:8 * (D + 1)].rearrange(
                "p (h j) -> p h j", j=D + 1)[:, :, 0:D]
            nc.vector.tensor_tensor(
                vo3a, ps[:, 0:512].rearrange("p (h j) -> p h j", j=D),
                bv[:, 0:512].rearrange("p (h j) -> p h j", j=D), op=OP.add)

        def emit_v_c(tc16):
            """Last 4 heads' V for one 128-token chunk."""
            ps = pse.tile([128, 1024], FP32, tag="pse")
            for ec in range(EC):
                nc.tensor.matmul(
                    ps[:, 0:256],
                    xt[ec][:, tc16 * 128:(tc16 + 1) * 128],
                    wsb[ec][:, 2 * E + 512:3 * E],
                    start=(ec == 0), stop=(ec == EC - 1))
            vo3b = vo[tc16][:, 8 * (D + 1):].rearrange(
                "p (h j) -> p h j", j=D + 1)[:, :, 0:D]
            nc.vector.tensor_tensor(
                vo3b, ps[:, 0:256].rearrange("p (h j) -> p h j", j=D),
                bv[:, 512:768].rearrange("p (h j) -> p h j", j=D), op=OP.add)

        # ---- attention inner pieces -------------------------------------
        et_pool = stack.enter_context(tc.tile_pool(name="et", bufs=14))
        small_pool = stack.enter_context(tc.tile_pool(name="small", bufs=1))
        rb_pool = stack.enter_context(tc.tile_pool(name="rb", bufs=2))
        osb_pool = stack.enter_context(tc.tile_pool(name="osb", bufs=4))

        dbl = [dbl_pool.tile([128, HM], BF16, tag=f"dbl{b}", name=f"dbl{b}")
               for b in range(BPC)]

        def alloc_pos():
            return [pso.tile([65, 512], FP32, tag="po", name=f"po{h}")
                    for h in range(2)]

        def emit_attnv_tk(st, pos, tk):
            b, fci, tq, ets = st
            for half in range(2):
                h = 2 * fci + half
                nc.tensor.matmul(
                    pos[half][:, :],
                    vo[b * 8 + tk][:, h * (D + 1):(h + 1) * (D + 1)],
                    ets[tk][:, half * 512:(half + 1) * 512],
                    start=(tk == 0), stop=(tk == 7))

        def emit_drain(st, pos):
            b, fci, tq, _ = st
            for half in range(2):
                h = 2 * fci + half
                po = pos[half]
                sraw = small_pool.tile([1, 512], FP32, tag="sraw")
                nc.vector.tensor_copy(sraw[:, :], po[D:D + 1, :])
                rec = small_pool.tile([1, 512], FP32, tag="rec")
                nc.vector.reciprocal_approx_fast(rec[:, :], sraw[:, :])
                rb = rb_pool.tile([64, 512], FP32, tag="rb")
                nc.gpsimd.partition_broadcast(rb[:, :], rec[:, :], channels=64)
                m0 = h * N + tq * 512
                nc.vector.tensor_tensor(
                    dbl[b][0:D, m0:m0 + 512], po[0:D, :], rb[:, :], op=OP.mult)
                if m0 == 0:
                    nc.vector.tensor_tensor(
                        dbl[b][D:128, 0:511], po[0:D, 1:512], rb[:, 1:512],
                        op=OP.mult)
                else:
                    nc.vector.tensor_tensor(
                        dbl[b][D:128, m0 - 1:m0 + 511], po[0:D, :], rb[:, :],
                        op=OP.mult)


        def emit_outproj_chunk(b, npc):
            pf = pse.tile([128, 1024], FP32, tag="pse")
            for cc in range(EC):
                off = 2 * cc + 12 * (npc * 128)
                lhsT = dbl[b][:, off::12][:, 0:128]
                nc.tensor.matmul(pf[:, 0:512], lhsT, wosb[cc][:, 0:512],
                                 start=(cc == 0), stop=(cc == EC - 1))
            for cc in range(EC):
                off = 2 * cc + 12 * (npc * 128)
                lhsT = dbl[b][:, off::12][:, 0:128]
                nc.tensor.matmul(pf[:, 512:768], lhsT, wosb[cc][:, 512:768],
                                 start=(cc == 0), stop=(cc == EC - 1))
            osb = osb_pool.tile([128, E], FP32, tag="osb")
            nc.vector.tensor_tensor(osb[:, :], pf[:, 0:768], bo[:, :], op=OP.add)
            # batch-1's last chunks go out on the scalar ring (Act is done
            # with exps by then and the sync/gpsimd rings are backlogged)
            oeng = ((nc.sync, nc.gpsimd)[npc % 2] if not (b == 1 and npc >= 4)
                    else (nc.scalar, nc.sync, nc.gpsimd, nc.scalar)[npc - 4])
            oeng.dma_start(
                out_ap[b * N + npc * 128:b * N + (npc + 1) * 128, :], osb[:, :])

        # ---- deferred-work scheduler ------------------------------------
        # Items: (due_sub, avail_sub, seq, thunk). Emitted in the attention
        # loop's tk slots; force-drained at subgroup start when overdue.
        work = []
        seq = [0]

        def add(due, avail, thunk):
            work.append([due, avail, seq[0], thunk])
            seq[0] += 1

        def sub_idx(b, fci, tq):
            return b * 12 + fci * 2 + tq

        # QK groups (prefix emits K0(b0) both windows + Q0(b0,0) directly).
        # avail = s0-4 keeps the 3-deep q/k tile-slot rotation safe: the
        # slot's previous tenant (fci-3) is fully read by then, so the DVE
        # bias write never blocks on future PE reads (which would deadlock
        # the 2-buf psum pool).
        for b in range(BPC):
            for fci in range(6):
                for kind in (1, 0):
                    for tql in range(2):
                        if b == 0 and fci == 0 and tql == 0:
                            continue  # in prefix
                        # due one subgroup before first use, so the DVE bias
                        # write has completed before the energy reads it
                        due = max(0, sub_idx(b, fci, tql if kind == 0 else 0) - 1)
                        avail = max(0, sub_idx(b, fci, 0) - 4)
                        add(due, avail, (lambda f=fci, k=kind, bb=b, t=tql:
                                         emit_qk_psum(f, k, bb, t)))
        # V chunks: heads 0-7 due at first attn@V of the batch; heads 8-11
        # due when fci=4 attn@V runs.
        for tc16 in range(T // 128):
            b = tc16 // 8
            add(sub_idx(b, 0, 1), max(0, sub_idx(b, 0, 1) - 6),
                (lambda t=tc16: emit_v_a(t)))
            add(sub_idx(b, 4, 1), max(0, sub_idx(b, 4, 1) - 6),
                (lambda t=tc16: emit_v_c(t)))
        # Out-projection: the scrambled (H,N,D)->(N,E) reshape is a pure
        # reinterpretation, so out-proj chunk npc reads only flat indices
        # npc*98304..+98304 = heads 1.5npc..1.5(npc+1). Its last-needed
        # drain (fci,tq) gives avail = (emission sub of that drain) + 1,
        # per-npc offsets [3,5,6,7,9,11,12,13] within the batch. Chunks
        # stream through the attention phase as low-priority filler; only
        # npc7 of batch 1 (needs the final drain) is left for the tail.
        # Subs 22/23 drain inline (one sub earlier than the prev-chain
        # formula), hence the b1 overrides for npc6/npc7.
        # NOTE: dependency-wise each chunk npc only needs drains up to
        # (fci,tq) = offsets [3,5,6,7,9,11,12,13] into its batch (the
        # scrambled reshape maps chunk npc to heads 1.5npc..1.5(npc+1)),
        # but spreading chunks through the attention phase measurably drops
        # the PE clock (454ns vs 379ns per 512-col matmul) — bursty
        # placement wins. batch-0 after its last drains (subs 13/14);
        # batch-1 npc0-3 fill sub 23, npc4-7 run in the tail (their
        # matmuls don't wait on the final drain).
        for npc in range(N // 128):
            add(23, 13 + npc // 4, (lambda n=npc: emit_outproj_chunk(0, n)))
        for npc in range(4):
            add(99, 23, (lambda n=npc: emit_outproj_chunk(1, n)))

        work.sort(key=lambda w: (w[0], w[2]))

        def pump(s, force_due=False, max_items=1):
            done = 0
            while done < max_items:
                best = None
                for w in work:
                    if w[1] > s:
                        continue
                    if force_due and w[0] > s:
                        break
                    best = w
                    break
                if best is None:
                    return done
                work.remove(best)
                best[3]()
                done += 1
            return done

        # ---- prefix (K w0 + Q w0 suffice for energy tk 0-3; K0 w1 is a
        # due-0 work item so its x-window-1 dependency doesn't stall sub 0)
        emit_qk_psum(0, 1, 0, 0)   # K0 b0 w0
        emit_qk_psum(0, 0, 0, 0)   # Q0 b0 w0

        # ---- attention loop (b-major) -----------------------------------
        # The last two subgroups (22, 23) run their own attn@V inline with a
        # lag of 2 behind the exp stream (instead of pipelined into the next
        # subgroup), so their drains land a subgroup earlier and the batch-1
        # out-projection overlaps the final exps.
        prev = None
        for b in range(BPC):
            for fci in range(H // 2):
                for tq in range(2):
                    s = sub_idx(b, fci, tq)
                    inline = (s >= 22)
                    pump(s, force_due=True, max_items=99)
                    ets = []
                    prev_pos = alloc_pos() if prev is not None else None
                    cur_pos = alloc_pos() if inline else None
                    for tk in range(8):
                        pe = pse.tile([128, 1024], FP32, tag="pse")
                        for half in range(2):
                            lo = 64 * half
                            nc.tensor.matmul(
                                pe[:, half * 512:(half + 1) * 512],
                                ktiles[(b, fci)][lo:lo + 64,
                                                 tk * 128:(tk + 1) * 128],
                                qtiles[(b, fci)][lo:lo + 64,
                                                 tq * 512:(tq + 1) * 512],
                                start=True, stop=True)
                        et = et_pool.tile([128, 1024], BF16, tag="et")
                        nc.scalar.activation(et[:, :], pe[:, :], AF.Exp,
                                             bias=0.0, scale=SCALE)
                        ets.append(et)
                        if prev is not None:
                            emit_attnv_tk(prev, prev_pos, tk)
                        if inline and tk >= 2:
                            emit_attnv_tk((b, fci, tq, ets), cur_pos, tk - 2)
                        if tk % 2:
                            pump(s)
                    if prev is not None:
                        emit_drain(prev, prev_pos)
                    if inline:
                        cur = (b, fci, tq, ets)
                        emit_attnv_tk(cur, cur_pos, 6)
                        emit_attnv_tk(cur, cur_pos, 7)
                        emit_drain(cur, cur_pos)
                        prev = None
                    else:
                        prev = (b, fci, tq, ets)

        # ---- tail --------------------------------------------------------
        while pump(24, max_items=1):
            pass
        # batch-1 out-proj npc 4..7: emitted after the final drain, but only
        # npc7's lhsT actually waits on it (heads 10-11 tq1).
        for npc in range(4, N // 128):
            emit_outproj_chunk(1, npc)


_built = None


def _build():
    global _built
    if _built is not None:
        return _built
    nc = bacc.Bacc("TRN2", target_bir_lowering=False, debug=False,
                   num_devices=N_CORES)
    x_ap = nc.dram_tensor("x", (E, T), BF16, kind="ExternalInput").ap()
    wqkv_ap = nc.dram_tensor("w_qkv", (E, F3), BF16, kind="ExternalInput").ap()
    bqkv_ap = nc.dram_tensor("b_qkv", (F3,), FP32, kind="ExternalInput").ap()
    wout_ap = nc.dram_tensor("w_out", (E, E), BF16, kind="ExternalInput").ap()
    bout_ap = nc.dram_tensor("b_out", (E,), FP32, kind="ExternalInput").ap()
    out_ap = nc.dram_tensor("out", (T, E), FP32, kind="ExternalOutput").ap()
    with tile.TileContext(nc) as tc:
        _emit(tc, x_ap, wqkv_ap, bqkv_ap, wout_ap, bout_ap, out_ap)
    nc.compile()
    _built = nc
    return nc


def kernel(x, W_qkv, b_qkv, W_out, b_out, _trace=False):
    import ml_dtypes
    x = np.asarray(x, dtype=np.float32).astype(ml_dtypes.bfloat16)
    xT = [np.ascontiguousarray(
        x[c * BPC:(c + 1) * BPC].reshape(T, E).T) for c in range(N_CORES)]
    W_qkv = np.ascontiguousarray(
        np.asarray(W_qkv, dtype=np.float32).astype(ml_dtypes.bfloat16))
    b_qkv = np.ascontiguousarray(np.asarray(b_qkv, dtype=np.float32))
    W_out = np.ascontiguousarray(
        np.asarray(W_out, dtype=np.float32).astype(ml_dtypes.bfloat16))
    b_out = np.ascontiguousarray(np.asarray(b_out, dtype=np.float32))

    nc = _build()
    in_maps = [
        {
            "x": xT[c],
            "w_qkv": W_qkv, "b_qkv": b_qkv, "w_out": W_out, "b_out": b_out,
        }
        for c in range(N_CORES)
    ]
    res = bass_utils.run_bass_kernel_spmd(
        nc, in_maps, core_ids=list(range(N_CORES)), trace=_trace)
    out = np.concatenate(
        [res.results[c]["out"].reshape(BPC, N, E) for c in range(N_CORES)],
        axis=0)
    if _trace:
        kernel._last_results = res
    return out


# revision 48
# speedup vs baseline: 1.0058x; 1.0058x over previous
"""Multi-head attention (B=16, N=1024, E=768, H=12) on 8 TRN2 NeuronCores.

Data parallel over batch (2 per core, no collectives). Per-core fused kernel
with a "left-shifted" schedule (~349us vs the 375us phase-serial baseline):
  - The PE is the global bottleneck (~306us busy incl overheads; the Act
    engine's softmax Exp stream is ~225us). The baseline ran QKV then
    attention mostly serially (first exp at ~90us); here the exp stream
    starts at ~25us: the prefix emits only K0/Q0 window-0 psum groups, and
    ALL other projection work (46 QK groups, V chunks, batch-0 out-proj)
    becomes deadline-scheduled filler popped into the attention loop's tk
    slots, keeping the PE dense while Act's ~40us of slack absorbs the
    resulting exp-stream jitter.
  - Keeping the PE *dense* matters beyond overlap: gaps early in the run
    leave the PE at a reduced clock (512-col matmul 454ns vs 379ns);
    chunky fillers + warmup matmuls hold the full clock.
  - Subgroup loop is batch-major so batch-0's out-projection overlaps
    batch-1's attention. The last two subgroups run their own attn@V
    inline (lag 2 behind the exp stream), which pulls the final drains a
    subgroup earlier and leaves only ~12us of out-proj + DMA tail.
  - DMA rings are ~100GB/s each and issue ~600ns apiece, serialized per
    engine: prefix-critical loads (x batch-0 window-0 halves, K0/Q0
    strips) lead the sync/scalar rings; late-needed loads trail. Issuing
    dma_start from inside the compute loop RACES (nondeterministic
    corruption) - all DMAs are issued up front, ordered per ring.
Math/layouts per head match the baseline bit-for-bit (energy pairs as
concurrent row-tiled matmuls, exp with scale=1/8 and no max-subtraction,
[V|1] lhsT for fused denominators, shifted-duplicate DOUBLE layout for the
scrambled (H,N,D)->(N,E) reshape feeding clean K=128 out-proj matmuls).
"""

import contextlib

import numpy as np

import concourse.bass as bass
import concourse.tile as tile
import concourse.mybir as mybir
from concourse import bacc
from concourse import bass_utils

B, N, E, H = 16, 1024, 768, 12
D = E // H          # 64
N_CORES = 8
BPC = B // N_CORES  # 2
T = BPC * N         # 2048
F3 = 3 * E
SCALE = 1.0 / float(np.sqrt(np.float32(D)))

FP32 = mybir.dt.float32
BF16 = mybir.dt.bfloat16
AF = mybir.ActivationFunctionType
OP = mybir.AluOpType


def _emit(tc, x_ap, wqkv_ap, bqkv_ap, wout_ap, bout_ap, out_ap):
    nc = tc.nc
    EC = E // 128      # 6
    FC = 2 * E // 128  # 12
    HM = H * N         # 12288

    stack = contextlib.ExitStack()
    with stack:
        const_pool = stack.enter_context(tc.tile_pool(name="const", bufs=1))
        w_pool = stack.enter_context(tc.tile_pool(name="w", bufs=1))
        qkt_pool = stack.enter_context(tc.tile_pool(name="qkt", bufs=1))
        vo_pool = stack.enter_context(tc.tile_pool(name="vo", bufs=1))
        dbl_pool = stack.enter_context(tc.tile_pool(name="dbl", bufs=1))
        xt_pool = stack.enter_context(tc.tile_pool(name="xt", bufs=1))
        warm_pool = stack.enter_context(tc.tile_pool(name="warm", bufs=1))

        pse = stack.enter_context(
            tc.tile_pool(name="pse", bufs=2, space="PSUM"))   # (128,1024) = 2 banks
        pso = stack.enter_context(
            tc.tile_pool(name="pso", bufs=4, space="PSUM"))   # (65,512) = 1 bank

        # ---- PE warmup during DMA wait (p-state ramp ~3us). Emitted first
        # so the memset is the DVE queue's first op and the matmuls start
        # immediately (they have no DMA deps).
        warm = warm_pool.tile([128, 512], BF16, tag="warm")
        nc.vector.memset(warm[:, :], 0.0)
        for _ in range(6):
            wps = pse.tile([128, 1024], FP32, tag="pse")
            nc.tensor.matmul(wps[:, 0:512], warm[:, 0:128], warm[:, :],
                             start=True, stop=True)

        # ---- weight / x tiles -------------------------------------------
        wsb = [w_pool.tile([128, F3], BF16, tag=f"wsb{ec}", name=f"wsb{ec}")
               for ec in range(EC)]
        wosb = [w_pool.tile([128, E], BF16, tag=f"wosb{ec}", name=f"wosb{ec}")
                for ec in range(EC)]
        xt = [xt_pool.tile([128, T], BF16, tag=f"xt{ec}", name=f"xt{ec}")
              for ec in range(EC)]

        # Critical-path DMAs: the prefix only needs x window 0 (cols 0:512)
        # plus the K0/Q0 strips, so x batch 0 is loaded in 512-col halves.
        # scalar ring: K0/K1 strips, then Q0/Q1 strips.
        for ec in range(EC):
            nc.scalar.dma_start(wsb[ec][:, 768:1024],
                                wqkv_ap[ec * 128:(ec + 1) * 128, 768:1024])
        for ec in range(EC):
            nc.scalar.dma_start(wsb[ec][:, 0:256],
                                wqkv_ap[ec * 128:(ec + 1) * 128, 0:256])
        # sync ring: x batch-0 halves (window 0 first — gates the prefix).
        for ec in range(EC):
            nc.sync.dma_start(xt[ec][:, 0:512], x_ap[ec * 128:(ec + 1) * 128, 0:512])
        for ec in range(EC):
            nc.sync.dma_start(xt[ec][:, 512:N],
                              x_ap[ec * 128:(ec + 1) * 128, 512:N])
        # gpsimd: bias constants.
        bq = const_pool.tile([128, FC], FP32, tag="bq")
        nc.gpsimd.dma_start(bq[:, :], bqkv_ap.rearrange("(c p) -> p c", p=128)[:, 0:FC])
        bv_row = const_pool.tile([1, E], FP32, tag="brow", name="bv_row")
        nc.gpsimd.dma_start(bv_row[:, :], bqkv_ap[2 * E:3 * E].unsqueeze(0))
        bv = const_pool.tile([128, E], FP32, tag="bv")
        nc.gpsimd.partition_broadcast(bv[:, :], bv_row[:, :], channels=128)
        bo_row = const_pool.tile([1, E], FP32, tag="brow", name="bo_row")
        nc.gpsimd.dma_start(bo_row[:, :], bout_ap.unsqueeze(0))
        bo = const_pool.tile([128, E], FP32, tag="bo")
        nc.gpsimd.partition_broadcast(bo[:, :], bo_row[:, :], channels=128)
        # sync, in transfer-priority order (per-queue transfers run in issue
        # order, so late-needed loads go last and don't contend for HBM
        # early): V weights (due ~sub1), Q2-5/K2-5 strips (due sub 4+),
        # x batch 1 (due ~sub 8), out-proj weights (due ~sub 13).
        for ec in range(EC):
            nc.sync.dma_start(wsb[ec][:, 1536:2304],
                              wqkv_ap[ec * 128:(ec + 1) * 128, 1536:2304])
        for ec in range(EC):
            nc.sync.dma_start(wsb[ec][:, 256:768],
                              wqkv_ap[ec * 128:(ec + 1) * 128, 256:768])
            nc.sync.dma_start(wsb[ec][:, 1024:1536],
                              wqkv_ap[ec * 128:(ec + 1) * 128, 1024:1536])
        for ec in range(EC):
            nc.sync.dma_start(xt[ec][:, N:T], x_ap[ec * 128:(ec + 1) * 128, N:T])
        for ec in range(EC):
            nc.sync.dma_start(wosb[ec][:, :],
                              wout_ap[ec * 128:(ec + 1) * 128, :])

        # ---- Q/K production ---------------------------------------------
        qtiles, ktiles = {}, {}

        def emit_qk_psum(fci, kind, b, tql):
            """One psum group: Q or K chunk fci, batch b, 512-token window."""
            tiles = qtiles if kind == 0 else ktiles
            key = (b, fci)
            if key not in tiles:
                tiles[key] = qkt_pool.tile(
                    [128, N], BF16, tag=f"{'qk'[kind]}{fci % 3}",
                    name=f"{'qk'[kind]}t{b}_{fci}")
            fc = fci + 6 * kind
            tch = 2 * b + tql
            ps = pse.tile([128, 1024], FP32, tag="pse")
            for ec in range(EC):
                nc.tensor.matmul(
                    ps[:, 0:512],
                    wsb[ec][:, fc * 128:(fc + 1) * 128],
                    xt[ec][:, tch * 512:(tch + 1) * 512],
                    start=(ec == 0), stop=(ec == EC - 1))
            nc.vector.tensor_scalar_add(
                tiles[key][:, tql * 512:(tql + 1) * 512], ps[:, 0:512],
                bq[:, fc:fc + 1])

        # ---- V chunks (tok-major, ones col per head) --------------------
        vo = [vo_pool.tile([128, H * (D + 1)], BF16, tag=f"vo{i}",
                           name=f"vo{i}") for i in range(T // 128)]

        def emit_v_a(tc16):
            """First 8 heads' V for one 128-token chunk (+ ones cols)."""
            ps = pse.tile([128, 1024], FP32, tag="pse")
            for ec in range(EC):
                nc.tensor.matmul(
                    ps[:, 0:512],
                    xt[ec][:, tc16 * 128:(tc16 + 1) * 128],
                    wsb[ec][:, 2 * E:2 * E + 512],
                    start=(ec == 0), stop=(ec == EC - 1))
            nc.vector.memset(vo[tc16][:, D::(D + 1)], 1.0)
            vo3a = vo[tc16][:, 0:8 * (D + 1)].rearrange(
                "p (h j) -> p h j", j=D + 1)[:, :, 0:D]
            nc.vector.tensor_tensor(
                vo3a, ps[:, 0:512].rearrange("p (h j) -> p h j", j=D),
                bv[:, 0:512].rearrange("p (h j) -> p h j", j=D), op=OP.add)

        def emit_v_c(tc16):
            """Last 4 heads' V for one 128-token chunk."""
            ps = pse.tile([128, 1024], FP32, tag="pse")
            for ec in range(EC):
                nc.tensor.matmul(
                    ps[:, 0:256],
                    xt[ec][:, tc16 * 128:(tc16 + 1) * 128],
                    wsb[ec][:, 2 * E + 512:3 * E],
                    start=(ec == 0), stop=(ec == EC - 1))
            vo3b = vo[tc16][:, 8 * (D + 1):].rearrange(
                "p (h j) -> p h j", j=D + 1)[:, :, 0:D]
            nc.vector.tensor_tensor(
                vo3b, ps[:, 0:256].rearrange("p (h j) -> p h j", j=D),
                bv[:, 512:768].rearrange("p (h j) -> p h j", j=D), op=OP.add)

        # ---- attention inner pieces -------------------------------------
        et_pool = stack.enter_context(tc.tile_pool(name="et", bufs=14))
        small_pool = stack.enter_context(tc.tile_pool(name="small", bufs=1))
        rb_pool = stack.enter_context(tc.tile_pool(name="rb", bufs=2))
        osb_pool = stack.enter_context(tc.tile_pool(name="osb", bufs=2))

        dbl = [dbl_pool.tile([128, HM], BF16, tag=f"dbl{b}", name=f"dbl{b}")
               for b in range(BPC)]

        def alloc_pos():
            return [pso.tile([65, 512], FP32, tag="po", name=f"po{h}")
                    for h in range(2)]

        def emit_attnv_tk(st, pos, tk):
            b, fci, tq, ets = st
            for half in range(2):
                h = 2 * fci + half
                nc.tensor.matmul(
                    pos[half][:, :],
                    vo[b * 8 + tk][:, h * (D + 1):(h + 1) * (D + 1)],
                    ets[tk][:, half * 512:(half + 1) * 512],
                    start=(tk == 0), stop=(tk == 7))

        def emit_drain(st, pos):
            b, fci, tq, _ = st
            for half in range(2):
                h = 2 * fci + half
                po = pos[half]
                sraw = small_pool.tile([1, 512], FP32, tag="sraw")
                nc.vector.tensor_copy(sraw[:, :], po[D:D + 1, :])
                rec = small_pool.tile([1, 512], FP32, tag="rec")
                nc.vector.reciprocal_approx_fast(rec[:, :], sraw[:, :])
                rb = rb_pool.tile([64, 512], FP32, tag="rb")
                nc.gpsimd.partition_broadcast(rb[:, :], rec[:, :], channels=64)
                m0 = h * N + tq * 512
                nc.vector.tensor_tensor(
                    dbl[b][0:D, m0:m0 + 512], po[0:D, :], rb[:, :], op=OP.mult)
                if m0 == 0:
                    nc.vector.tensor_tensor(
                        dbl[b][D:128, 0:511], po[0:D, 1:512], rb[:, 1:512],
                        op=OP.mult)
                else:
                    nc.vector.tensor_tensor(
                        dbl[b][D:128, m0 - 1:m0 + 511], po[0:D, :], rb[:, :],
                        op=OP.mult)

        def emit_outproj_chunk(b, npc):
            pf = pse.tile([128, 1024], FP32, tag="pse")
            for cc in range(EC):
                off = 2 * cc + 12 * (npc * 128)
                lhsT = dbl[b][:, off::12][:, 0:128]
                nc.tensor.matmul(pf[:, 0:512], lhsT, wosb[cc][:, 0:512],
                                 start=(cc == 0), stop=(cc == EC - 1))
            for cc in range(EC):
                off = 2 * cc + 12 * (npc * 128)
                lhsT = dbl[b][:, off::12][:, 0:128]
                nc.tensor.matmul(pf[:, 512:768], lhsT, wosb[cc][:, 512:768],
                                 start=(cc == 0), stop=(cc == EC - 1))
            osb = osb_pool.tile([128, E], FP32, tag="osb")
            nc.vector.tensor_tensor(osb[:, :], pf[:, 0:768], bo[:, :], op=OP.add)
            # batch-1's last chunks go out on the scalar ring (Act is done
            # with exps by then and the sync/gpsimd rings are backlogged)
            oeng = ((nc.sync, nc.gpsimd)[npc % 2] if not (b == 1 and npc >= 4)
                    else (nc.scalar, nc.sync, nc.gpsimd, nc.scalar)[npc - 4])
            oeng.dma_start(
                out_ap[b * N + npc * 128:b * N + (npc + 1) * 128, :], osb[:, :])

        # ---- deferred-work scheduler ------------------------------------
        # Items: (due_sub, avail_sub, seq, thunk). Emitted in the attention
        # loop's tk slots; force-drained at subgroup start when overdue.
        work = []
        seq = [0]

        def add(due, avail, thunk):
            work.append([due, avail, seq[0], thunk])
            seq[0] += 1

        def sub_idx(b, fci, tq):
            return b * 12 + fci * 2 + tq

        # QK groups (prefix emits K0(b0) both windows + Q0(b0,0) directly).
        # avail = s0-4 keeps the 3-deep q/k tile-slot rotation safe: the
        # slot's previous tenant (fci-3) is fully read by then, so the DVE
        # bias write never blocks on future PE reads (which would deadlock
        # the 2-buf psum pool).
        for b in range(BPC):
            for fci in range(6):
                for kind in (1, 0):
                    for tql in range(2):
                        if b == 0 and fci == 0 and tql == 0:
                            continue  # in prefix
                        # due one subgroup before first use, so the DVE bias
                        # write has completed before the energy reads it
                        due = max(0, sub_idx(b, fci, tql if kind == 0 else 0) - 1)
                        avail = max(0, sub_idx(b, fci, 0) - 4)
                        add(due, avail, (lambda f=fci, k=kind, bb=b, t=tql:
                                         emit_qk_psum(f, k, bb, t)))
        # V chunks: heads 0-7 due at first attn@V of the batch; heads 8-11
        # due when fci=4 attn@V runs.
        for tc16 in range(T // 128):
            b = tc16 // 8
            add(sub_idx(b, 0, 1), max(0, sub_idx(b, 0, 1) - 6),
                (lambda t=tc16: emit_v_a(t)))
            add(sub_idx(b, 4, 1), max(0, sub_idx(b, 4, 1) - 6),
                (lambda t=tc16: emit_v_c(t)))
        # Out-projection: the scrambled (H,N,D)->(N,E) reshape is a pure
        # reinterpretation, so out-proj chunk npc reads only flat indices
        # npc*98304..+98304 = heads 1.5npc..1.5(npc+1). Its last-needed
        # drain (fci,tq) gives avail = (emission sub of that drain) + 1,
        # per-npc offsets [3,5,6,7,9,11,12,13] within the batch. Chunks
        # stream through the attention phase as low-priority filler; only
        # npc7 of batch 1 (needs the final drain) is left for the tail.
        # Subs 22/23 drain inline (one sub earlier than the prev-chain
        # formula), hence the b1 overrides for npc6/npc7.
        # NOTE: dependency-wise each chunk npc only needs drains up to
        # (fci,tq) = offsets [3,5,6,7,9,11,12,13] into its batch (the
        # scrambled reshape maps chunk npc to heads 1.5npc..1.5(npc+1)),
        # but spreading chunks through the attention phase measurably drops
        # the PE clock (454ns vs 379ns per 512-col matmul) — bursty
        # placement wins. batch-0 after its last drains (subs 13/14);
        # batch-1 npc0-3 fill sub 23, npc4-7 run in the tail (their
        # matmuls don't wait on the final drain).
        for npc in range(N // 128):
            add(23, 13 + npc // 4, (lambda n=npc: emit_outproj_chunk(0, n)))
        for npc in range(4):
            add(99, 23, (lambda n=npc: emit_outproj_chunk(1, n)))

        work.sort(key=lambda w: (w[0], w[2]))

        def pump(s, force_due=False, max_items=1):
            done = 0
            while done < max_items:
                best = None
                for w in work:
                    if w[1] > s:
                        continue
                    if force_due and w[0] > s:
                        break
                    best = w
                    break
                if best is None:
                    return done
                work.remove(best)
                best[3]()
                done += 1
            return done

        # ---- prefix (K w0 + Q w0 suffice for energy tk 0-3; K0 w1 is a
        # due-0 work item so its x-window-1 dependency doesn't stall sub 0)
        emit_qk_psum(0, 1, 0, 0)   # K0 b0 w0
        emit_qk_psum(0, 0, 0, 0)   # Q0 b0 w0

        # ---- attention loop (b-major) -----------------------------------
        # The last two subgroups (22, 23) run their own attn@V inline with a
        # lag of 2 behind the exp stream (instead of pipelined into the next
        # subgroup), so their drains land a subgroup earlier and the batch-1
        # out-projection overlaps the final exps.
        prev = None
        for b in range(BPC):
            for fci in range(H // 2):
                for tq in range(2):
                    s = sub_idx(b, fci, tq)
                    inline = (s >= 22)
                    pump(s, force_due=True, max_items=99)
                    ets = []
                    prev_pos = alloc_pos() if prev is not None else None
                    cur_pos = alloc_pos() if inline else None
                    for tk in range(8):
                        pe = pse.tile([128, 1024], FP32, tag="pse")
                        for half in range(2):
                            lo = 64 * half
                            nc.tensor.matmul(
                                pe[:, half * 512:(half + 1) * 512],
                                ktiles[(b, fci)][lo:lo + 64,
                                                 tk * 128:(tk + 1) * 128],
                                qtiles[(b, fci)][lo:lo + 64,
                                                 tq * 512:(tq + 1) * 512],
                                start=True, stop=True)
                        et = et_pool.tile([128, 1024], BF16, tag="et")
                        nc.scalar.activation(et[:, :], pe[:, :], AF.Exp,
                                             bias=0.0, scale=SCALE)
                        ets.append(et)
                        if prev is not None:
                            emit_attnv_tk(prev, prev_pos, tk)
                        if inline and tk >= 2:
                            emit_attnv_tk((b, fci, tq, ets), cur_pos, tk - 2)
                        if tk % 2:
                            pump(s)
                    if prev is not None:
                        emit_drain(prev, prev_pos)
                    if inline:
                        cur = (b, fci, tq, ets)
                        emit_attnv_tk(cur, cur_pos, 6)
                        emit_attnv_tk(cur, cur_pos, 7)
                        emit_drain(cur, cur_pos)
                        prev = None
                    else:
                        prev = (b, fci, tq, ets)

        # ---- tail --------------------------------------------------------
        while pump(24, max_items=1):
            pass
        # batch-1 out-proj npc 4..7: emitted after the final drain, but only
        # npc7's lhsT actually waits on it (heads 10-11 tq1).
        for npc in range(4, N // 128):
            emit_outproj_chunk(1, npc)


_built = None


def _build():
    global _built
    if _built is not None:
        return _built
    nc = bacc.Bacc("TRN2", target_bir_lowering=False, debug=False,
                   num_devices=N_CORES)
    x_ap = nc.dram_tensor("x", (E, T), BF16, kind="ExternalInput").ap()
    wqkv_ap = nc.dram_tensor("w_qkv", (E, F3), BF16, kind="ExternalInput").ap()
    bqkv_ap = nc.dram_tensor("b_qkv", (F3,), FP32, kind="ExternalInput").ap()
    wout_ap = nc.dram_tensor("w_out", (E, E), BF16, kind="ExternalInput").ap()
    bout_ap = nc.dram_tensor("b_out", (E,), FP32, kind="ExternalInput").ap()
    out_ap = nc.dram_tensor("out", (T, E), FP32, kind="ExternalOutput").ap()
    with tile.TileContext(nc) as tc:
        _emit(tc, x_ap, wqkv_ap, bqkv_ap, wout_ap, bout_ap, out_ap)
    nc.compile()
    _built = nc
    return nc


def kernel(x, W_qkv, b_qkv, W_out, b_out, _trace=False):
    import ml_dtypes
    x = np.asarray(x, dtype=np.float32).astype(ml_dtypes.bfloat16)
    xT = [np.ascontiguousarray(
        x[c * BPC:(c + 1) * BPC].reshape(T, E).T) for c in range(N_CORES)]
    W_qkv = np.ascontiguousarray(
        np.asarray(W_qkv, dtype=np.float32).astype(ml_dtypes.bfloat16))
    b_qkv = np.ascontiguousarray(np.asarray(b_qkv, dtype=np.float32))
    W_out = np.ascontiguousarray(
        np.asarray(W_out, dtype=np.float32).astype(ml_dtypes.bfloat16))
    b_out = np.ascontiguousarray(np.asarray(b_out, dtype=np.float32))

    nc = _build()
    in_maps = [
        {
            "x": xT[c],
            "w_qkv": W_qkv, "b_qkv": b_qkv, "w_out": W_out, "b_out": b_out,
        }
        for c in range(N_CORES)
    ]
    res = bass_utils.run_bass_kernel_spmd(
        nc, in_maps, core_ids=list(range(N_CORES)), trace=_trace)
    out = np.concatenate(
        [res.results[c]["out"].reshape(BPC, N, E) for c in range(N_CORES)],
        axis=0)
    if _trace:
        kernel._last_results = res
    return out
